# revision 18
# baseline (speedup 1.0000x reference)
"""Trainium2 Bass kernel for nn_ChildHAggregation (gnn_message_passing).

Per-sample math:
  x = [hl, hr]; q/k from HyperLinear(h, xh); 2x2 softmax attention;
  x += scores @ [hl, hr]; layernorm(ddof=1, alpha/beta);
  out = hyper(x; hU,hWu,hWb) + hyper(xw; lU,lWu,lWb)

Strategy (v1, data-parallel over 8 cores, 4096 rows/core, 32 tiles of 128):
  - Host: cast hl/hr/xh/xw to fp16; fold all weight/bias combinations in
    fp64 numpy (qWb' = qWb + qWu*diag(qU_b); WC = hWb + hWu*diag(bh) +
    lWb + lWu*diag(lU_b); hUa = alpha (.) hU; cs = colsum(hUa); cb, qb
    combined bias rows); pre-arrange weights chunk-major [128, c, 512].
  - Device: NO PE transposes. Transposed input tiles come from the DMA
    XBAR transpose (fp16, out[p,c,s] = X[s, c*128+p]) straight from
    DRAM; y_top/y_bot mixes are transposed SBUF->SBUF the same way.
  - 2-token softmax == sigmoid of score differences; attention + layer-
    norm fold into per-sample scalars. Hidden path uses the y-trick:
    y_t = inv*(a_t*hl + b_t*hr) formed sample-major (vector+scalar),
    DMA-transposed, so x@hUa collapses to 2 matmul units instead of 4.
  - Superblocks of C=8 tiles, two passes (A: scores+stats+y; D: output
    matmuls) so PSUM never exceeds 8 banks and the PE stream is gap-free
    (PE p-state ramps to 2.4 GHz only when continuously busy).
  - All matmul operands fp16 (1 cyc/row, ~5e-4 rel err vs 2e-2 budget).
"""

import numpy as np
from contextlib import ExitStack

import concourse.bacc as bacc
import concourse.bass as bass
import concourse.mybir as mybir
import concourse.tile as tile
from concourse.bass_utils import run_bass_kernel_spmd

N_CORES = 8
B_FULL = 32768
HALF = 512
DIM = 1024
P = 128
C = 8  # tiles per superblock
EPS = 1e-6
INV_SQRT_HALF = 1.0 / float(np.sqrt(np.float32(HALF)))

f32 = mybir.dt.float32
f16 = mybir.dt.float16

AX = mybir.AxisListType
ALU = mybir.AluOpType
ACTF = mybir.ActivationFunctionType

W4 = ["qU", "kU", "qWu", "qWb", "kWu", "hWu", "lWu", "WC"]
W8 = ["hUa", "lU"]
BROWS = ["qWu_b", "kWu_b", "qb", "hWu_b", "lWu_b", "cb", "cs"]


def build_nc(b_loc, c_sb=C, tap=None):
    """tap: optional name of an intermediate to stream to the 'tap' output
    (per tile, [P, *] f32) for debugging."""
    n_tiles = b_loc // P
    assert n_tiles % c_sb == 0

    nc = bacc.Bacc("TRN2", target_bir_lowering=False, debug=False,
                   num_devices=1)

    d = {}
    for nm in ["hl16", "hr16", "xh16"]:
        d[nm] = nc.dram_tensor(nm, [b_loc, HALF], f16,
                               kind="ExternalInput").ap()
    d["xw16"] = nc.dram_tensor("xw16", [b_loc, DIM], f16,
                               kind="ExternalInput").ap()
    for w in W4:
        d[w] = nc.dram_tensor(w, [P, 4, HALF], f16, kind="ExternalInput").ap()
    for w in W8:
        d[w] = nc.dram_tensor(w, [P, 8, HALF], f16, kind="ExternalInput").ap()
    for b in BROWS:
        d[b] = nc.dram_tensor(b, [HALF], f32, kind="ExternalInput").ap()
    out_d = nc.dram_tensor("out", [b_loc, HALF], f32,
                           kind="ExternalOutput").ap()
    tap_d = None
    if tap is not None:
        tap_d = nc.dram_tensor("tap", [b_loc, HALF], f32,
                               kind="ExternalOutput").ap()

    with tile.TileContext(nc) as tc, ExitStack() as ctx:
        # ---------- persistent weights / broadcast biases ----------
        wts = ctx.enter_context(tc.tile_pool(name="wts", bufs=1))
        wsb = {}
        # DMA order = first-use order
        for w in ["qWu", "qWb", "kWu", "qU", "kU", "hWu", "lWu", "WC"]:
            wsb[w] = wts.tile([P, 4, HALF], f16, name=f"w_{w}")
            nc.sync.dma_start(wsb[w], d[w])
        for w in W8:
            wsb[w] = wts.tile([P, 8, HALF], f16, name=f"w_{w}")
            nc.sync.dma_start(wsb[w], d[w])
        bc = {}
        rowp = ctx.enter_context(tc.tile_pool(name="rowp", bufs=1))
        for b in BROWS:
            r = rowp.tile([1, HALF], f32, name=f"row_{b}")
            nc.scalar.dma_start(r, d[b][None, :])
            bc[b] = wts.tile([P, HALF], f32, name=f"bc_{b}")
            nc.gpsimd.partition_broadcast(bc[b], r)
        from concourse.masks import make_identity
        ident32 = rowp.tile([P, P], f32, name="ident32")
        make_identity(nc, ident32)
        ident = wts.tile([P, P], f16, name="ident")
        nc.vector.tensor_copy(ident, ident32)

        # ---------- pools ----------
        tin = ctx.enter_context(tc.tile_pool(name="tin", bufs=3))
        hdp = ctx.enter_context(tc.tile_pool(name="hdp", bufs=2))
        sin = ctx.enter_context(tc.tile_pool(name="sin", bufs=3))
        per = ctx.enter_context(tc.tile_pool(name="per", bufs=c_sb + 1))
        pha = ctx.enter_context(tc.tile_pool(name="pha", bufs=2))
        scr = ctx.enter_context(tc.tile_pool(name="scr", bufs=2))
        tinyp = ctx.enter_context(tc.tile_pool(name="tinyp", bufs=2))
        yp = ctx.enter_context(tc.tile_pool(name="yp", bufs=2))
        phd = ctx.enter_context(tc.tile_pool(name="phd", bufs=2))
        outp = ctx.enter_context(tc.tile_pool(name="outp", bufs=2))
        psum = ctx.enter_context(tc.tile_pool(name="psum", bufs=6,
                                              space="PSUM"))
        tpps = ctx.enter_context(tc.tile_pool(name="tpps", bufs=2,
                                              space="PSUM"))

        perN = {}  # persistent per-tile handles keyed (idx mod (C+2))

        def do_tap(i, name, ap, width=HALF):
            """Stream an intermediate to tap_d for debugging."""
            if tap != name:
                return
            rs = bass.ts(i, P)
            t = outp.tile([P, HALF], f32, tag="tapt", name=f"tap_{i}")
            nc.vector.tensor_copy(t[:, :width], ap)
            nc.sync.dma_start(tap_d[rs, :width], t[:, :width])

        def passA(i):
            rs = bass.ts(i, P)
            # ---- input DMAs (plain, fp16 sample-major) ----
            hl_s = sin.tile([P, HALF], f16, tag="hls", name=f"hls_{i}")
            nc.sync.dma_start(hl_s, d["hl16"][rs, :])
            hr_s = sin.tile([P, HALF], f16, tag="hrs", name=f"hrs_{i}")
            nc.sync.dma_start(hr_s, d["hr16"][rs, :])
            xh_s = sin.tile([P, HALF], f16, tag="xhs", name=f"xhs_{i}")
            nc.scalar.dma_start(xh_s, d["xh16"][rs, :])
            xw_s = sin.tile([P, DIM], f16, tag="xws", name=f"xws_{i}")
            nc.scalar.dma_start(xw_s, d["xw16"][rs, :])

            # ---- PE transposes (fp16, 1 cyc/row) + evictions ----
            def pe_t(src, ncols, dst, dst_off=0):
                ps = tpps.tile([P, HALF], f16, tag="tp", name=f"tp_{i}_{id(src)}_{dst_off}")
                for c in range(ncols):
                    nc.tensor.transpose(ps[:, c * P:(c + 1) * P],
                                        src[:, (dst_off * 4 + c) * P:
                                            (dst_off * 4 + c + 1) * P], ident)
                nc.scalar.copy(dst[:, dst_off * HALF:dst_off * HALF + ncols * P],
                               ps[:, :ncols * P])
                return ps

            hlT = tin.tile([P, HALF], f16, tag="hlT", name=f"hlT_{i}")
            pe_t(hl_s, 4, hlT)
            hrT = tin.tile([P, HALF], f16, tag="hrT", name=f"hrT_{i}")
            pe_t(hr_s, 4, hrT)
            hdT = hdp.tile([P, HALF], f16, tag="hdT", name=f"hdT_{i}")
            nc.vector.tensor_sub(hdT, hlT, hrT)
            xhT = per.tile([P, HALF], f16, tag="xhT", name=f"xhT_{i}")
            pe_t(xh_s, 4, xhT)
            xwT = per.tile([P, DIM], f16, tag="xwT", name=f"xwT_{i}")
            pe_t(xw_s, 4, xwT, 0)
            pe_t(xw_s, 4, xwT, 1)

            if tap in ("hlT", "hdT"):
                tt = {"hlT": hlT, "hdT": hdT}[tap]
                t = outp.tile([P, HALF], f32, tag="tapt", name=f"tap_{i}")
                nc.vector.tensor_copy(t, tt)
                nc.sync.dma_start(tap_d[rs, :], t)

            # ---- matmuls: xh group first, then A_l/A_r/CD ----
            def unit(tag):
                return psum.tile([P, HALF], f32, tag="mm",
                                 name=f"ps_{tag}_{i}")

            SUq, SBq, TU = unit("SUq"), unit("SBq"), unit("TU")
            for c in range(4):
                lhs = xhT[:, bass.ts(c, P)]
                st, sp = (c == 0), (c == 3)
                nc.tensor.matmul(SUq, lhs, wsb["qWu"][:, c, :], start=st, stop=sp)
                nc.tensor.matmul(SBq, lhs, wsb["qWb"][:, c, :], start=st, stop=sp)
                nc.tensor.matmul(TU, lhs, wsb["kWu"][:, c, :], start=st, stop=sp)
            A_l, A_r, CD = unit("A_l"), unit("A_r"), unit("CD")
            for c in range(4):
                nc.tensor.matmul(A_l, hlT[:, bass.ts(c, P)], wsb["qU"][:, c, :],
                                 start=(c == 0), stop=(c == 3))
            for c in range(4):
                nc.tensor.matmul(A_r, hrT[:, bass.ts(c, P)], wsb["qU"][:, c, :],
                                 start=(c == 0), stop=(c == 3))
            for c in range(4):
                nc.tensor.matmul(CD, hdT[:, bass.ts(c, P)], wsb["kU"][:, c, :],
                                 start=(c == 0), stop=(c == 3))

            # ---- row stats (fp16 inputs; scalar engine takes sl/sr) ----
            sl = tinyp.tile([P, 1], f32, tag="sl")
            s0 = scr.tile([P, HALF], f16, tag="scr", name=f"scr_sl_{i}")
            nc.scalar.activation(s0, hl_s, ACTF.Copy, accum_out=sl)
            sr = tinyp.tile([P, 1], f32, tag="sr")
            s1 = scr.tile([P, HALF], f16, tag="scr", name=f"scr_sr_{i}")
            nc.scalar.activation(s1, hr_s, ACTF.Copy, accum_out=sr)
            ql = tinyp.tile([P, 1], f32, tag="ql")
            s2 = scr.tile([P, HALF], f16, tag="scr", name=f"scr_ql_{i}")
            nc.vector.scalar_tensor_tensor(
                s2, hl_s, 0.0, hl_s, ALU.bypass, ALU.mult, accum_out=ql)
            qr = tinyp.tile([P, 1], f32, tag="qr")
            s3 = scr.tile([P, HALF], f16, tag="scr", name=f"scr_qr_{i}")
            nc.vector.scalar_tensor_tensor(
                s3, hr_s, 0.0, hr_s, ALU.bypass, ALU.mult, accum_out=qr)
            cr2 = tinyp.tile([P, 1], f32, tag="cr2")
            s4 = scr.tile([P, HALF], f16, tag="scr", name=f"scr_cr_{i}")
            nc.vector.scalar_tensor_tensor(
                s4, hl_s, 0.0, hr_s, ALU.bypass, ALU.mult, accum_out=cr2)

            # ---- phase A epilogue ----
            su = pha.tile([P, HALF], f32, tag="su")
            nc.vector.tensor_add(su, SUq, bc["qWu_b"])
            tu = pha.tile([P, HALF], f32, tag="tu")
            nc.vector.tensor_add(tu, TU, bc["kWu_b"])
            sbq = pha.tile([P, HALF], f32, tag="sbq")
            nc.vector.tensor_add(sbq, SBq, bc["qb"])
            dk = pha.tile([P, HALF], f32, tag="dk")
            nc.vector.tensor_mul(dk, CD, tu)
            u = pha.tile([P, HALF], f32, tag="u")
            nc.vector.tensor_mul(u, su, dk)

            do_tap(i, "SUq", SUq)
            do_tap(i, "A_l", A_l)
            do_tap(i, "A_r", A_r)
            do_tap(i, "CD", CD)
            do_tap(i, "su", su)
            do_tap(i, "tu", tu)
            do_tap(i, "sbq", sbq)
            do_tap(i, "dk", dk)
            do_tap(i, "u", u)

            stats = tinyp.tile([P, 3], f32, tag="stats")
            for j, (aa, bb) in enumerate([(sbq, dk), (A_l, u), (A_r, u)]):
                sd = scr.tile([P, HALF], f16, tag="scr", name=f"scr_d{j}_{i}")
                nc.vector.scalar_tensor_tensor(
                    sd, aa, 0.0, bb, ALU.bypass, ALU.mult,
                    accum_out=stats[:, j:j + 1])

            # ---- 2-way softmax via sigmoid; per-sample scalar algebra ----
            diffs = tinyp.tile([P, 2], f32, tag="diffs")
            nc.vector.tensor_add(diffs, stats[:, 1:3],
                                 stats[:, 0:1].broadcast_to([P, 2]))
            probs = tinyp.tile([P, 2], f32, tag="probs")
            nc.scalar.activation(probs, diffs, ACTF.Sigmoid,
                                 scale=INV_SQRT_HALF)
            a0 = tinyp.tile([P, 1], f32, tag="a0")
            nc.scalar.activation(a0, probs[:, 0:1], ACTF.Copy, bias=1.0)
            b0 = tinyp.tile([P, 1], f32, tag="b0")
            nc.scalar.activation(b0, probs[:, 0:1], ACTF.Copy, scale=-1.0,
                                 bias=1.0)
            a1 = probs[:, 1:2]
            b1 = tinyp.tile([P, 1], f32, tag="b1")
            nc.scalar.activation(b1, probs[:, 1:2], ACTF.Copy, scale=-1.0,
                                 bias=2.0)

            e0 = tinyp.tile([P, 1], f32, tag="e0")
            nc.vector.tensor_add(e0, a0, a1)
            e1 = tinyp.tile([P, 1], f32, tag="e1")
            nc.vector.tensor_add(e1, b0, b1)
            sumx = tinyp.tile([P, 1], f32, tag="sumx")
            nc.vector.tensor_mul(sumx, sl, e0)
            nc.vector.scalar_tensor_tensor(sumx, sr, e1, sumx,
                                           ALU.mult, ALU.add)
            f0 = tinyp.tile([P, 1], f32, tag="f0")
            nc.vector.tensor_mul(f0, a0, a0)
            nc.vector.scalar_tensor_tensor(f0, a1, a1, f0, ALU.mult, ALU.add)
            f1 = tinyp.tile([P, 1], f32, tag="f1")
            nc.vector.tensor_mul(f1, b0, b0)
            nc.vector.scalar_tensor_tensor(f1, b1, b1, f1, ALU.mult, ALU.add)
            f2 = tinyp.tile([P, 1], f32, tag="f2")
            nc.vector.tensor_mul(f2, a0, b0)
            nc.vector.scalar_tensor_tensor(f2, a1, b1, f2, ALU.mult, ALU.add)
            nc.scalar.activation(f2, f2, ACTF.Copy, scale=2.0)
            ssq = tinyp.tile([P, 1], f32, tag="ssq")
            nc.vector.tensor_mul(ssq, ql, f0)
            nc.vector.scalar_tensor_tensor(ssq, qr, f1, ssq, ALU.mult, ALU.add)
            nc.vector.scalar_tensor_tensor(ssq, cr2, f2, ssq,
                                           ALU.mult, ALU.add)
            m2x = tinyp.tile([P, 1], f32, tag="m2x")
            nc.vector.tensor_mul(m2x, sumx, sumx)
            varn = tinyp.tile([P, 1], f32, tag="varn")
            nc.vector.scalar_tensor_tensor(varn, m2x, -1.0 / DIM, ssq,
                                           ALU.mult, ALU.add)
            stde = tinyp.tile([P, 1], f32, tag="stde")
            nc.scalar.activation(stde, varn, ACTF.Sqrt, scale=1.0 / (DIM - 1))
            nc.scalar.activation(stde, stde, ACTF.Copy, bias=EPS)
            rinv = tinyp.tile([P, 1], f32, tag="rinv")
            nc.vector.reciprocal(rinv, stde)
            mean = tinyp.tile([P, 1], f32, tag="mean")
            nc.scalar.activation(mean, sumx, ACTF.Copy, scale=1.0 / DIM)
            im = per.tile([P, 1], f32, tag="im", name=f"im_{i}")
            nc.vector.tensor_mul(im, mean, rinv)

            do_tap(i, "stats", stats, width=3)
            do_tap(i, "probs", probs, width=2)
            do_tap(i, "sl", sl, width=1)
            do_tap(i, "ql", ql, width=1)
            do_tap(i, "cr2", cr2, width=1)
            do_tap(i, "sumx", sumx, width=1)
            do_tap(i, "ssq", ssq, width=1)
            do_tap(i, "rinv", rinv, width=1)
            do_tap(i, "im", im, width=1)

            ya0 = tinyp.tile([P, 1], f32, tag="ya0")
            nc.vector.tensor_mul(ya0, a0, rinv)
            yb0 = tinyp.tile([P, 1], f32, tag="yb0")
            nc.vector.tensor_mul(yb0, b0, rinv)
            ya1 = tinyp.tile([P, 1], f32, tag="ya1")
            nc.vector.tensor_mul(ya1, a1, rinv)
            yb1 = tinyp.tile([P, 1], f32, tag="yb1")
            nc.vector.tensor_mul(yb1, b1, rinv)

            # ---- y mixes (sample-major), then XBAR transpose ----
            ty0 = yp.tile([P, HALF], f16, tag="ty0")
            nc.scalar.activation(ty0, hl_s, ACTF.Copy, scale=ya0)
            yt_s = yp.tile([P, HALF], f16, tag="yt")
            nc.vector.scalar_tensor_tensor(yt_s, hr_s, yb0, ty0,
                                           ALU.mult, ALU.add)
            ty1 = yp.tile([P, HALF], f16, tag="ty1")
            nc.scalar.activation(ty1, hl_s, ACTF.Copy, scale=ya1)
            yb_s = yp.tile([P, HALF], f16, tag="yb")
            nc.vector.scalar_tensor_tensor(yb_s, hr_s, yb1, ty1,
                                           ALU.mult, ALU.add)
            # y transposes via PE (fp16 identity matmul) + scalar evict
            yTt = per.tile([P, HALF], f16, tag="yTt", name=f"yTt_{i}")
            yTb = per.tile([P, HALF], f16, tag="yTb", name=f"yTb_{i}")
            for ksrc, dst in ((yt_s, yTt), (yb_s, yTb)):
                ps = tpps.tile([P, HALF], f16, tag="tp",
                               name=f"tpy_{i}_{id(dst)}")
                for c in range(4):
                    nc.tensor.transpose(ps[:, c * P:(c + 1) * P],
                                        ksrc[:, c * P:(c + 1) * P], ident)
                nc.scalar.copy(dst, ps)

            do_tap(i, "yt", yt_s)
            do_tap(i, "yb", yb_s)

            perN[i] = (xhT, xwT, yTt, yTb, im)

        def passD(i):
            rs = bass.ts(i, P)
            xhT, xwT, yTt, yTb, im = perN.pop(i)

            def unit(tag):
                return psum.tile([P, HALF], f32, tag="mm",
                                 name=f"ps_{tag}_{i}")

            HU = unit("HU")
            for cc in range(8):
                ysrc = yTt if cc < 4 else yTb
                nc.tensor.matmul(HU, ysrc[:, bass.ts(cc % 4, P)],
                                 wsb["hUa"][:, cc, :],
                                 start=(cc == 0), stop=(cc == 7))
            HSU, LSU, SBC = unit("HSU"), unit("LSU"), unit("SBC")
            for c in range(4):
                lhs = xhT[:, bass.ts(c, P)]
                st, sp = (c == 0), (c == 3)
                nc.tensor.matmul(HSU, lhs, wsb["hWu"][:, c, :], start=st, stop=sp)
                nc.tensor.matmul(LSU, lhs, wsb["lWu"][:, c, :], start=st, stop=sp)
                nc.tensor.matmul(SBC, lhs, wsb["WC"][:, c, :], start=st, stop=sp)
            LUp = unit("LU")
            for c in range(8):
                nc.tensor.matmul(LUp, xwT[:, bass.ts(c, P)], wsb["lU"][:, c, :],
                                 start=(c == 0), stop=(c == 7))

            # ---- epilogue: out = w1 + sbc - su_h*(cs*im - HU) ----
            t5 = phd.tile([P, HALF], f16, tag="t5")
            nc.vector.scalar_tensor_tensor(t5, bc["cs"], im, HU,
                                           ALU.mult, ALU.subtract)
            su_h = phd.tile([P, HALF], f16, tag="su_h")
            nc.vector.tensor_add(su_h, HSU, bc["hWu_b"])
            su_l = phd.tile([P, HALF], f16, tag="su_l")
            nc.vector.tensor_add(su_l, LSU, bc["lWu_b"])
            sbc = phd.tile([P, HALF], f16, tag="sbc")
            nc.vector.tensor_add(sbc, SBC, bc["cb"])
            w1 = phd.tile([P, HALF], f16, tag="w1")
            nc.vector.tensor_mul(w1, LUp, su_l)

            do_tap(i, "HU", HU)
            do_tap(i, "HSU", HSU)
            do_tap(i, "LUp", LUp)
            do_tap(i, "t5", t5)
            do_tap(i, "su_h", su_h)
            do_tap(i, "su_l", su_l)
            do_tap(i, "sbc", sbc)
            do_tap(i, "w1", w1)
            v1 = phd.tile([P, HALF], f16, tag="v1")
            nc.gpsimd.tensor_mul(v1, t5, su_h)
            acc = phd.tile([P, HALF], f16, tag="acc")
            nc.gpsimd.tensor_add(acc, w1, sbc)
            out_t = outp.tile([P, HALF], f32, tag="out_t")
            nc.gpsimd.tensor_sub(out_t, acc, v1)
            nc.sync.dma_start(out_d[rs, :], out_t)

        for s in range(n_tiles // c_sb):
            for t in range(c_sb):
                passA(s * c_sb + t)
            for t in range(c_sb):
                passD(s * c_sb + t)

    nc.compile()
    return nc


_NC_CACHE = {}


def _get_nc(b_loc):
    if b_loc not in _NC_CACHE:
        _NC_CACHE[b_loc] = build_nc(b_loc)
    return _NC_CACHE[b_loc]


def _prep_shared(inputs):
    """Host-side weight folding (fp64) + fp16 arrangement."""
    g = {k: np.asarray(v, dtype=np.float64) for k, v in inputs.items()}
    qb = g["qWb_b"] + g["qU_b"] * g["qWu_b"]
    qWb_f = g["qWb_w"] + g["qWu_w"] * g["qU_b"][None, :]
    bh = g["beta"] @ g["hU_w"] + g["hU_b"]
    WC = (g["hWb_w"] + g["hWu_w"] * bh[None, :]
          + g["lWb_w"] + g["lWu_w"] * g["lU_b"][None, :])
    cb = (g["hWb_b"] + bh * g["hWu_b"]
          + g["lWb_b"] + g["lU_b"] * g["lWu_b"])
    hUa = g["hU_w"] * g["alpha"][:, None]
    cs = hUa.sum(0)

    def arr(w, nch):
        return np.ascontiguousarray(
            w.reshape(nch, P, HALF).transpose(1, 0, 2)).astype(np.float16)

    shared = {
        "qU": arr(g["qU_w"], 4), "kU": arr(g["kU_w"], 4),
        "qWu": arr(g["qWu_w"], 4), "qWb": arr(qWb_f, 4),
        "kWu": arr(g["kWu_w"], 4), "hWu": arr(g["hWu_w"], 4),
        "lWu": arr(g["lWu_w"], 4), "WC": arr(WC, 4),
        "hUa": arr(hUa, 8), "lU": arr(g["lU_w"], 8),
        "qWu_b": g["qWu_b"].astype(np.float32),
        "kWu_b": g["kWu_b"].astype(np.float32),
        "qb": qb.astype(np.float32),
        "hWu_b": g["hWu_b"].astype(np.float32),
        "lWu_b": g["lWu_b"].astype(np.float32),
        "cb": cb.astype(np.float32), "cs": cs.astype(np.float32),
    }
    return shared


def prep_in_maps(inputs):
    b = inputs["hl"].shape[0]
    b_loc = b // N_CORES
    shared = _prep_shared(inputs)
    hl16 = np.asarray(inputs["hl"], dtype=np.float16)
    hr16 = np.asarray(inputs["hr"], dtype=np.float16)
    xh16 = np.asarray(inputs["xh"], dtype=np.float16)
    xw16 = np.asarray(inputs["xw"], dtype=np.float16)
    in_maps = []
    for i in range(N_CORES):
        sl = slice(i * b_loc, (i + 1) * b_loc)
        m = dict(shared)
        m["hl16"] = hl16[sl]
        m["hr16"] = hr16[sl]
        m["xh16"] = xh16[sl]
        m["xw16"] = xw16[sl]
        in_maps.append(m)
    return in_maps, b_loc


def kernel(**inputs):
    in_maps, b_loc = prep_in_maps(inputs)
    nc = _get_nc(b_loc)
    res = run_bass_kernel_spmd(nc, in_maps, core_ids=list(range(N_CORES)))
    return np.concatenate([r["out"] for r in res.results], axis=0)


# revision 20
# speedup vs baseline: 1.4890x; 1.4890x over previous
"""Trainium2 Bass kernel for nn_ChildHAggregation (gnn_message_passing).

Per-sample math:
  x = [hl, hr]; q/k from HyperLinear(h, xh); 2x2 softmax attention;
  x += scores @ [hl, hr]; layernorm(ddof=1, alpha/beta);
  out = hyper(x; hU,hWu,hWb) + hyper(xw; lU,lWu,lWb)

Strategy (v1, data-parallel over 8 cores, 4096 rows/core, 32 tiles of 128):
  - Host: cast hl/hr/xh/xw to fp16; fold all weight/bias combinations in
    fp64 numpy (qWb' = qWb + qWu*diag(qU_b); WC = hWb + hWu*diag(bh) +
    lWb + lWu*diag(lU_b); hUa = alpha (.) hU; cs = colsum(hUa); cb, qb
    combined bias rows); pre-arrange weights chunk-major [128, c, 512].
  - Device: NO PE transposes. Transposed input tiles come from the DMA
    XBAR transpose (fp16, out[p,c,s] = X[s, c*128+p]) straight from
    DRAM; y_top/y_bot mixes are transposed SBUF->SBUF the same way.
  - 2-token softmax == sigmoid of score differences; attention + layer-
    norm fold into per-sample scalars. Hidden path uses the y-trick:
    y_t = inv*(a_t*hl + b_t*hr) formed sample-major (vector+scalar),
    DMA-transposed, so x@hUa collapses to 2 matmul units instead of 4.
  - Superblocks of C=8 tiles, two passes (A: scores+stats+y; D: output
    matmuls) so PSUM never exceeds 8 banks and the PE stream is gap-free
    (PE p-state ramps to 2.4 GHz only when continuously busy).
  - All matmul operands fp16 (1 cyc/row, ~5e-4 rel err vs 2e-2 budget).
"""

import numpy as np
from contextlib import ExitStack

import concourse.bacc as bacc
import concourse.bass as bass
import concourse.mybir as mybir
import concourse.tile as tile
from concourse.bass_utils import run_bass_kernel_spmd

N_CORES = 8
B_FULL = 32768
HALF = 512
DIM = 1024
P = 128
C = 8  # tiles per superblock
EPS = 1e-6
INV_SQRT_HALF = 1.0 / float(np.sqrt(np.float32(HALF)))

f32 = mybir.dt.float32
f16 = mybir.dt.float16

AX = mybir.AxisListType
ALU = mybir.AluOpType
ACTF = mybir.ActivationFunctionType

W4 = ["qU", "kU", "qWu", "qWb", "kWu", "hWu", "lWu", "WC"]
W8 = ["hUa", "lU"]
BROWS = ["qWu_b", "kWu_b", "qb", "hWu_b", "lWu_b", "cb", "cs"]


def build_nc(b_loc, c_sb=C, tap=None):
    """tap: optional name of an intermediate to stream to the 'tap' output
    (per tile, [P, *] f32) for debugging."""
    n_tiles = b_loc // P
    assert n_tiles % c_sb == 0

    nc = bacc.Bacc("TRN2", target_bir_lowering=False, debug=False,
                   num_devices=1)

    d = {}
    for nm in ["hl16", "hr16", "xh16"]:
        d[nm] = nc.dram_tensor(nm, [b_loc, HALF], f16,
                               kind="ExternalInput").ap()
    d["xw16"] = nc.dram_tensor("xw16", [b_loc, DIM], f16,
                               kind="ExternalInput").ap()
    for w in W4:
        d[w] = nc.dram_tensor(w, [P, 4, HALF], f16, kind="ExternalInput").ap()
    for w in W8:
        d[w] = nc.dram_tensor(w, [P, 8, HALF], f16, kind="ExternalInput").ap()
    for b in BROWS:
        d[b] = nc.dram_tensor(b, [HALF], f32, kind="ExternalInput").ap()
    out_d = nc.dram_tensor("out", [b_loc, HALF], f32,
                           kind="ExternalOutput").ap()
    tap_d = None
    if tap is not None:
        tap_d = nc.dram_tensor("tap", [b_loc, HALF], f32,
                               kind="ExternalOutput").ap()

    with tile.TileContext(nc) as tc, ExitStack() as ctx:
        # ---------- persistent weights / broadcast biases ----------
        wts = ctx.enter_context(tc.tile_pool(name="wts", bufs=1))
        wsb = {}
        # DMA order = first-use order
        for w in ["qWu", "qWb", "kWu", "qU", "kU", "hWu", "lWu", "WC"]:
            wsb[w] = wts.tile([P, 4, HALF], f16, name=f"w_{w}")
            nc.sync.dma_start(wsb[w], d[w])
        for w in W8:
            wsb[w] = wts.tile([P, 8, HALF], f16, name=f"w_{w}")
            nc.sync.dma_start(wsb[w], d[w])
        bc = {}
        rowp = ctx.enter_context(tc.tile_pool(name="rowp", bufs=1))
        for b in BROWS:
            r = rowp.tile([1, HALF], f32, name=f"row_{b}")
            nc.scalar.dma_start(r, d[b][None, :])
            bc[b] = wts.tile([P, HALF], f32, name=f"bc_{b}")
            nc.gpsimd.partition_broadcast(bc[b], r)
        from concourse.masks import make_identity
        ident32 = rowp.tile([P, P], f32, name="ident32")
        make_identity(nc, ident32)
        ident = wts.tile([P, P], f16, name="ident")
        nc.vector.tensor_copy(ident, ident32)

        # ---------- pools ----------
        tin = ctx.enter_context(tc.tile_pool(name="tin", bufs=3))
        hdp = ctx.enter_context(tc.tile_pool(name="hdp", bufs=2))
        sin = ctx.enter_context(tc.tile_pool(name="sin", bufs=3))
        per = ctx.enter_context(tc.tile_pool(name="per", bufs=c_sb + 1))
        pha = ctx.enter_context(tc.tile_pool(name="pha", bufs=2))
        scr = ctx.enter_context(tc.tile_pool(name="scr", bufs=2))
        tinyp = ctx.enter_context(tc.tile_pool(name="tinyp", bufs=2))
        yp = ctx.enter_context(tc.tile_pool(name="yp", bufs=2))
        phd = ctx.enter_context(tc.tile_pool(name="phd", bufs=2))
        outp = ctx.enter_context(tc.tile_pool(name="outp", bufs=2))
        psum = ctx.enter_context(tc.tile_pool(name="psum", bufs=6,
                                              space="PSUM"))
        tpps = ctx.enter_context(tc.tile_pool(name="tpps", bufs=2,
                                              space="PSUM"))

        perN = {}  # persistent per-tile handles keyed (idx mod (C+2))

        def do_tap(i, name, ap, width=HALF):
            """Stream an intermediate to tap_d for debugging."""
            if tap != name:
                return
            rs = bass.ts(i, P)
            t = outp.tile([P, HALF], f32, tag="tapt", name=f"tap_{i}")
            nc.vector.tensor_copy(t[:, :width], ap)
            nc.sync.dma_start(tap_d[rs, :width], t[:, :width])

        pending_y = []  # deferred y-transpose emitters, (tile_idx, fn)
        tp_ctr = [0]

        def flush_y(upto=None):
            while pending_y and (upto is None or pending_y[0][0] <= upto):
                pending_y.pop(0)[1]()

        def passA(i):
            rs = bass.ts(i, P)
            # ---- input DMAs (plain, fp16 sample-major) ----
            hl_s = sin.tile([P, HALF], f16, tag="hls", name=f"hls_{i}")
            nc.sync.dma_start(hl_s, d["hl16"][rs, :])
            hr_s = sin.tile([P, HALF], f16, tag="hrs", name=f"hrs_{i}")
            nc.sync.dma_start(hr_s, d["hr16"][rs, :])
            xh_s = sin.tile([P, HALF], f16, tag="xhs", name=f"xhs_{i}")
            nc.scalar.dma_start(xh_s, d["xh16"][rs, :])
            xw_s = sin.tile([P, DIM], f16, tag="xws", name=f"xws_{i}")
            nc.scalar.dma_start(xw_s, d["xw16"][rs, :])

            # ---- PE transposes; two groups share one PSUM bank, single
            # wide eviction per bank keeps the scalar queue short ----
            def tp_bank():
                tp_ctr[0] += 1
                return tpps.tile([P, 2 * HALF], f16, tag="tp",
                                 name=f"tp_{tp_ctr[0]}")

            def pe_t(ps, src, src_off, ncols, ps_off):
                for c in range(ncols):
                    nc.tensor.transpose(
                        ps[:, ps_off + c * P:ps_off + (c + 1) * P],
                        src[:, (src_off + c) * P:(src_off + c + 1) * P], ident)

            psA = tp_bank()
            pe_t(psA, hl_s, 0, 4, 0)
            pe_t(psA, hr_s, 0, 4, HALF)
            hlrT = tin.tile([P, 2 * HALF], f16, tag="hlrT", name=f"hlrT_{i}")
            nc.scalar.copy(hlrT, psA)
            hlT = hlrT[:, :HALF]
            hrT = hlrT[:, HALF:]

            psB = tp_bank()
            pe_t(psB, xh_s, 0, 4, 0)
            pe_t(psB, xw_s, 0, 4, HALF)
            xT = per.tile([P, 3 * HALF], f16, tag="xT", name=f"xT_{i}")
            nc.scalar.copy(xT[:, :2 * HALF], psB)

            psC = tp_bank()
            pe_t(psC, xw_s, 4, 4, 0)
            nc.scalar.copy(xT[:, 2 * HALF:], psC[:, :HALF])
            xhT = xT[:, :HALF]
            xwT = xT[:, HALF:]

            hdT = hdp.tile([P, HALF], f16, tag="hdT", name=f"hdT_{i}")
            nc.vector.tensor_sub(hdT, hlT, hrT)

            if tap in ("hlT", "hdT"):
                tt = {"hlT": hlT, "hdT": hdT}[tap]
                t = outp.tile([P, HALF], f32, tag="tapt", name=f"tap_{i}")
                nc.vector.tensor_copy(t, tt)
                nc.sync.dma_start(tap_d[rs, :], t)

            # ---- matmuls: xh group first, then A_l/A_r/CD ----
            def unit(tag):
                return psum.tile([P, HALF], f32, tag="mm",
                                 name=f"ps_{tag}_{i}")

            SUq, SBq, TU = unit("SUq"), unit("SBq"), unit("TU")
            for c in range(4):
                lhs = xhT[:, bass.ts(c, P)]
                st, sp = (c == 0), (c == 3)
                nc.tensor.matmul(SUq, lhs, wsb["qWu"][:, c, :], start=st, stop=sp)
                nc.tensor.matmul(SBq, lhs, wsb["qWb"][:, c, :], start=st, stop=sp)
                nc.tensor.matmul(TU, lhs, wsb["kWu"][:, c, :], start=st, stop=sp)
            A_l, A_r, CD = unit("A_l"), unit("A_r"), unit("CD")
            for c in range(4):
                nc.tensor.matmul(A_l, hlT[:, bass.ts(c, P)], wsb["qU"][:, c, :],
                                 start=(c == 0), stop=(c == 3))
            for c in range(4):
                nc.tensor.matmul(A_r, hrT[:, bass.ts(c, P)], wsb["qU"][:, c, :],
                                 start=(c == 0), stop=(c == 3))
            for c in range(4):
                nc.tensor.matmul(CD, hdT[:, bass.ts(c, P)], wsb["kU"][:, c, :],
                                 start=(c == 0), stop=(c == 3))

            # y transposes of tile i-2 go here in PE program order: their
            # epilogue scalars are long since ready, so the PE never waits.
            flush_y(i - 2)

            # ---- row stats (DVE 16-bit, accum reductions) ----
            def vstat(nm, in0, s1, in1, op0, op1):
                t = tinyp.tile([P, 1], f32, tag=nm)
                s = scr.tile([P, HALF], f16, tag="scr", name=f"scr_{nm}_{i}")
                nc.vector.scalar_tensor_tensor(s, in0, s1, in1, op0, op1,
                                               accum_out=t)
                return t

            sl = vstat("sl", hl_s, 0.0, hl_s, ALU.bypass, ALU.bypass)
            sr = vstat("sr", hr_s, 0.0, hr_s, ALU.bypass, ALU.bypass)
            ql = vstat("ql", hl_s, 0.0, hl_s, ALU.bypass, ALU.mult)
            qr = vstat("qr", hr_s, 0.0, hr_s, ALU.bypass, ALU.mult)
            # cr2 pre-doubled: accumulate (2*hl)*hr
            cr2 = vstat("cr2", hl_s, 2.0, hr_s, ALU.mult, ALU.mult)

            # ---- phase A epilogue ----
            su = pha.tile([P, HALF], f32, tag="su")
            nc.vector.tensor_add(su, SUq, bc["qWu_b"])
            tu = pha.tile([P, HALF], f32, tag="tu")
            nc.vector.tensor_add(tu, TU, bc["kWu_b"])
            sbq = pha.tile([P, HALF], f32, tag="sbq")
            nc.vector.tensor_add(sbq, SBq, bc["qb"])
            dk = pha.tile([P, HALF], f32, tag="dk")
            nc.vector.tensor_mul(dk, CD, tu)
            u = pha.tile([P, HALF], f32, tag="u")
            nc.vector.tensor_mul(u, su, dk)

            do_tap(i, "SUq", SUq)
            do_tap(i, "su", su)
            do_tap(i, "dk", dk)

            stats = tinyp.tile([P, 3], f32, tag="stats")
            for j, (aa, bb) in enumerate([(sbq, dk), (A_l, u), (A_r, u)]):
                sd = scr.tile([P, HALF], f16, tag="scr", name=f"scr_d{j}_{i}")
                nc.vector.scalar_tensor_tensor(
                    sd, aa, 0.0, bb, ALU.bypass, ALU.mult,
                    accum_out=stats[:, j:j + 1])

            # ---- 2-way softmax via sigmoid; per-sample scalar algebra ----
            diffs = tinyp.tile([P, 2], f32, tag="diffs")
            nc.vector.tensor_add(diffs, stats[:, 1:3],
                                 stats[:, 0:1].broadcast_to([P, 2]))
            probs = tinyp.tile([P, 2], f32, tag="probs")
            nc.scalar.activation(probs, diffs, ACTF.Sigmoid,
                                 scale=INV_SQRT_HALF)
            a0 = tinyp.tile([P, 1], f32, tag="a0")
            nc.vector.tensor_scalar_add(a0, probs[:, 0:1], 1.0)
            b0 = tinyp.tile([P, 1], f32, tag="b0")
            nc.vector.tensor_scalar(b0, probs[:, 0:1], -1.0, 1.0,
                                    ALU.mult, ALU.add)
            a1 = probs[:, 1:2]
            b1 = tinyp.tile([P, 1], f32, tag="b1")
            nc.vector.tensor_scalar(b1, probs[:, 1:2], -1.0, 2.0,
                                    ALU.mult, ALU.add)

            e0 = tinyp.tile([P, 1], f32, tag="e0")
            nc.vector.tensor_add(e0, a0, a1)
            e1 = tinyp.tile([P, 1], f32, tag="e1")
            nc.vector.tensor_add(e1, b0, b1)
            sumx = tinyp.tile([P, 1], f32, tag="sumx")
            nc.vector.tensor_mul(sumx, sl, e0)
            nc.vector.scalar_tensor_tensor(sumx, sr, e1, sumx,
                                           ALU.mult, ALU.add)
            f0 = tinyp.tile([P, 1], f32, tag="f0")
            nc.vector.tensor_mul(f0, a0, a0)
            nc.vector.scalar_tensor_tensor(f0, a1, a1, f0, ALU.mult, ALU.add)
            f1 = tinyp.tile([P, 1], f32, tag="f1")
            nc.vector.tensor_mul(f1, b0, b0)
            nc.vector.scalar_tensor_tensor(f1, b1, b1, f1, ALU.mult, ALU.add)
            f2 = tinyp.tile([P, 1], f32, tag="f2")
            nc.vector.tensor_mul(f2, a0, b0)
            nc.vector.scalar_tensor_tensor(f2, a1, b1, f2, ALU.mult, ALU.add)
            ssq = tinyp.tile([P, 1], f32, tag="ssq")
            nc.vector.tensor_mul(ssq, ql, f0)
            nc.vector.scalar_tensor_tensor(ssq, qr, f1, ssq, ALU.mult, ALU.add)
            nc.vector.scalar_tensor_tensor(ssq, cr2, f2, ssq,
                                           ALU.mult, ALU.add)
            m2x = tinyp.tile([P, 1], f32, tag="m2x")
            nc.vector.tensor_mul(m2x, sumx, sumx)
            varn = tinyp.tile([P, 1], f32, tag="varn")
            nc.vector.scalar_tensor_tensor(varn, m2x, -1.0 / DIM, ssq,
                                           ALU.mult, ALU.add)
            stde = tinyp.tile([P, 1], f32, tag="stde")
            nc.scalar.activation(stde, varn, ACTF.Sqrt, scale=1.0 / (DIM - 1))
            seps = tinyp.tile([P, 1], f32, tag="seps")
            nc.vector.tensor_scalar_add(seps, stde, EPS)
            rinv = tinyp.tile([P, 1], f32, tag="rinv")
            nc.vector.reciprocal(rinv, seps)
            mean = tinyp.tile([P, 1], f32, tag="mean")
            nc.vector.tensor_scalar_mul(mean, sumx, 1.0 / DIM)
            im = per.tile([P, 1], f32, tag="im", name=f"im_{i}")
            nc.vector.tensor_mul(im, mean, rinv)

            ya0 = tinyp.tile([P, 1], f32, tag="ya0")
            nc.vector.tensor_mul(ya0, a0, rinv)
            yb0 = tinyp.tile([P, 1], f32, tag="yb0")
            nc.vector.tensor_mul(yb0, b0, rinv)
            ya1 = tinyp.tile([P, 1], f32, tag="ya1")
            nc.vector.tensor_mul(ya1, a1, rinv)
            yb1 = tinyp.tile([P, 1], f32, tag="yb1")
            nc.vector.tensor_mul(yb1, b1, rinv)

            do_tap(i, "stats", stats, width=3)
            do_tap(i, "probs", probs, width=2)
            do_tap(i, "rinv", rinv, width=1)
            do_tap(i, "im", im, width=1)

            # ---- y mixes (sample-major, fp16) ----
            ty0 = yp.tile([P, HALF], f16, tag="ty0")
            nc.vector.tensor_scalar_mul(ty0, hl_s, ya0)
            yt_s = yp.tile([P, HALF], f16, tag="yt", name=f"yt_{i}")
            nc.vector.scalar_tensor_tensor(yt_s, hr_s, yb0, ty0,
                                           ALU.mult, ALU.add)
            ty1 = yp.tile([P, HALF], f16, tag="ty1")
            nc.vector.tensor_scalar_mul(ty1, hl_s, ya1)
            yb_s = yp.tile([P, HALF], f16, tag="yb", name=f"yb_{i}")
            nc.vector.scalar_tensor_tensor(yb_s, hr_s, yb1, ty1,
                                           ALU.mult, ALU.add)

            do_tap(i, "yt", yt_s)

            yT = per.tile([P, 2 * HALF], f16, tag="yT", name=f"yT_{i}")

            def emit_y(yt_s=yt_s, yb_s=yb_s, yT=yT, i=i):
                psY = tp_bank()
                pe_t(psY, yt_s, 0, 4, 0)
                pe_t(psY, yb_s, 0, 4, HALF)
                nc.scalar.copy(yT, psY)

            pending_y.append((i, emit_y))
            perN[i] = (xT, yT, im)

        def passD(i):
            rs = bass.ts(i, P)
            xT, yT, im = perN.pop(i)
            xhT = xT[:, :HALF]
            xwT = xT[:, HALF:]
            yTt = yT[:, :HALF]
            yTb = yT[:, HALF:]

            def unit(tag):
                return psum.tile([P, HALF], f32, tag="mm",
                                 name=f"ps_{tag}_{i}")

            HU = unit("HU")
            for cc in range(8):
                ysrc = yTt if cc < 4 else yTb
                nc.tensor.matmul(HU, ysrc[:, bass.ts(cc % 4, P)],
                                 wsb["hUa"][:, cc, :],
                                 start=(cc == 0), stop=(cc == 7))
            HSU, LSU, SBC = unit("HSU"), unit("LSU"), unit("SBC")
            for c in range(4):
                lhs = xhT[:, bass.ts(c, P)]
                st, sp = (c == 0), (c == 3)
                nc.tensor.matmul(HSU, lhs, wsb["hWu"][:, c, :], start=st, stop=sp)
                nc.tensor.matmul(LSU, lhs, wsb["lWu"][:, c, :], start=st, stop=sp)
                nc.tensor.matmul(SBC, lhs, wsb["WC"][:, c, :], start=st, stop=sp)
            LUp = unit("LU")
            for c in range(8):
                nc.tensor.matmul(LUp, xwT[:, bass.ts(c, P)], wsb["lU"][:, c, :],
                                 start=(c == 0), stop=(c == 7))

            if i % c_sb == 0:
                flush_y()  # boundary tiles' y transposes, now safely ready

            # ---- epilogue: out = w1 + sbc - su_h*(cs*im - HU) ----
            t5 = phd.tile([P, HALF], f16, tag="t5")
            nc.vector.scalar_tensor_tensor(t5, bc["cs"], im, HU,
                                           ALU.mult, ALU.subtract)
            su_h = phd.tile([P, HALF], f16, tag="su_h")
            nc.vector.tensor_add(su_h, HSU, bc["hWu_b"])
            su_l = phd.tile([P, HALF], f16, tag="su_l")
            nc.vector.tensor_add(su_l, LSU, bc["lWu_b"])
            sbc = phd.tile([P, HALF], f16, tag="sbc")
            nc.vector.tensor_add(sbc, SBC, bc["cb"])
            w1 = phd.tile([P, HALF], f16, tag="w1")
            nc.vector.tensor_mul(w1, LUp, su_l)

            do_tap(i, "HU", HU)
            do_tap(i, "t5", t5)
            do_tap(i, "su_h", su_h)
            do_tap(i, "w1", w1)

            v1 = phd.tile([P, HALF], f16, tag="v1")
            nc.gpsimd.tensor_mul(v1, t5, su_h)
            acc = phd.tile([P, HALF], f16, tag="acc")
            nc.gpsimd.tensor_add(acc, w1, sbc)
            out_t = outp.tile([P, HALF], f32, tag="out_t")
            nc.gpsimd.tensor_sub(out_t, acc, v1)
            nc.sync.dma_start(out_d[rs, :], out_t)

        for s in range(n_tiles // c_sb):
            for t in range(c_sb):
                passA(s * c_sb + t)
            for t in range(c_sb):
                passD(s * c_sb + t)

    nc.compile()
    return nc


_NC_CACHE = {}


def _get_nc(b_loc):
    if b_loc not in _NC_CACHE:
        _NC_CACHE[b_loc] = build_nc(b_loc)
    return _NC_CACHE[b_loc]


def _prep_shared(inputs):
    """Host-side weight folding (fp64) + fp16 arrangement."""
    g = {k: np.asarray(v, dtype=np.float64) for k, v in inputs.items()}
    qb = g["qWb_b"] + g["qU_b"] * g["qWu_b"]
    qWb_f = g["qWb_w"] + g["qWu_w"] * g["qU_b"][None, :]
    bh = g["beta"] @ g["hU_w"] + g["hU_b"]
    WC = (g["hWb_w"] + g["hWu_w"] * bh[None, :]
          + g["lWb_w"] + g["lWu_w"] * g["lU_b"][None, :])
    cb = (g["hWb_b"] + bh * g["hWu_b"]
          + g["lWb_b"] + g["lU_b"] * g["lWu_b"])
    hUa = g["hU_w"] * g["alpha"][:, None]
    cs = hUa.sum(0)

    def arr(w, nch):
        return np.ascontiguousarray(
            w.reshape(nch, P, HALF).transpose(1, 0, 2)).astype(np.float16)

    shared = {
        "qU": arr(g["qU_w"], 4), "kU": arr(g["kU_w"], 4),
        "qWu": arr(g["qWu_w"], 4), "qWb": arr(qWb_f, 4),
        "kWu": arr(g["kWu_w"], 4), "hWu": arr(g["hWu_w"], 4),
        "lWu": arr(g["lWu_w"], 4), "WC": arr(WC, 4),
        "hUa": arr(hUa, 8), "lU": arr(g["lU_w"], 8),
        "qWu_b": g["qWu_b"].astype(np.float32),
        "kWu_b": g["kWu_b"].astype(np.float32),
        "qb": qb.astype(np.float32),
        "hWu_b": g["hWu_b"].astype(np.float32),
        "lWu_b": g["lWu_b"].astype(np.float32),
        "cb": cb.astype(np.float32), "cs": cs.astype(np.float32),
    }
    return shared


def prep_in_maps(inputs):
    b = inputs["hl"].shape[0]
    b_loc = b // N_CORES
    shared = _prep_shared(inputs)
    hl16 = np.asarray(inputs["hl"], dtype=np.float16)
    hr16 = np.asarray(inputs["hr"], dtype=np.float16)
    xh16 = np.asarray(inputs["xh"], dtype=np.float16)
    xw16 = np.asarray(inputs["xw"], dtype=np.float16)
    in_maps = []
    for i in range(N_CORES):
        sl = slice(i * b_loc, (i + 1) * b_loc)
        m = dict(shared)
        m["hl16"] = hl16[sl]
        m["hr16"] = hr16[sl]
        m["xh16"] = xh16[sl]
        m["xw16"] = xw16[sl]
        in_maps.append(m)
    return in_maps, b_loc


def kernel(**inputs):
    in_maps, b_loc = prep_in_maps(inputs)
    nc = _get_nc(b_loc)
    res = run_bass_kernel_spmd(nc, in_maps, core_ids=list(range(N_CORES)))
    return np.concatenate([r["out"] for r in res.results], axis=0)
